# revision 1
# baseline (speedup 1.0000x reference)
"""Trainium2 Bass kernel for the spike-decoder GNN message-passing module.

Math (per batch b, output time tau in [0, T-2], variable v):
  out[b,tau,v] = bias[v]
               + sum_{i,k} w[v,i,k] * x[b,i,tau+k-(K-2)]          (static conv)
               + sum_{e: recv[e]=v} sum_k dw[e,b,tau,k] * x[b,send[e],tau+k-(K-2)]
with w = conv_weight masked at w[i,i,K-1] = 0, x = spikes[...,0] transposed to
[b, nvar, t], and out-of-range x treated as zero.

Sharding: 8 cores = (b in 0..3) x (time half h in 0..1). Each core computes a
1024-wide tau window ([0,1024) or [1023,2047) — one overlapping column keeps
shapes uniform for SPMD). dyn_weights is the only big tensor (268 MB); its
[E, 1024, K] slice per core is the memory-bound stream.

On-core algorithm (all fp32):
  - xg[e,:] = x[send[e],:] gathered via one-hot matmul on PE (exact: x is 0/1)
  - products P[e,(tau,k)] = dw_tile * sliding-window(xg) on DVE (one big
    tensor_tensor per e-tile with an overlapping stride-1 AP for the window)
  - k-reduction + recv-scatter + transpose folded into PE: for each k, a
    matmul with stationary one-hot recv matrix and moving operand = strided
    columns P[:, tau*K+k], accumulating into PSUM[v, tau]
  - static conv: 16 matmuls with stationary wT_k and shifted xpad slices
  - bias: rank-1 matmul (bias x ones)
All terms accumulate into one PSUM bank [v, 512], copied out by ScalarE.
Output is [v, tau] per core; host transposes while assembling the result.
"""

import numpy as np

B, T, NVAR, K, E = 4, 2048, 128, 16, 512
TAU = T - 1            # 2047
L = 1024               # per-core tau window
NC_COUNT = 8
W_XPAD = L + K         # 1040 (1039 used; padded even for f32r matmul ISA)
ETILES = E // 128      # 4
CHUNK = 512            # tau chunk per PSUM bank
NCHUNK = L // CHUNK    # 2

_PROGRAM = None


def _build_program():
    import concourse.bass as bass
    import concourse.bacc as bacc
    import concourse.mybir as mybir
    import concourse.tile as tile

    f32 = mybir.dt.float32
    # float32r: same fp32 bytes, but the PE streams 1 row/cycle (vs 4 for
    # strict fp32 which needs 2 half-rate passes) when the moving dim >= 256.
    f32r = mybir.dt.float32r
    bf16 = mybir.dt.bfloat16
    # Bacc (not plain Bass): its compile pipeline runs generate_event_semaphores,
    # which splits multi-semaphore waits — a raw fp32 Matmult supports only one
    # sync-wait slot and walrus rejects more ("Too many sync wait commands").
    nc = bacc.Bacc()

    xpad_d = nc.declare_dram_parameter("xpad", [NVAR, W_XPAD], f32r, isOutput=False)
    dw_d = nc.declare_dram_parameter("dw", [NCHUNK * E, CHUNK * K], f32, isOutput=False)
    ssend_d = nc.declare_dram_parameter("ssend", [NVAR, E], f32r, isOutput=False)
    wt_d = nc.declare_dram_parameter("wt", [NVAR, K * NVAR], f32r, isOutput=False)
    recv_d = nc.declare_dram_parameter("recvT", [128, ETILES * NVAR], bf16, isOutput=False)
    bo_d = nc.declare_dram_parameter("bias_ones", [1, NVAR + CHUNK], f32r, isOutput=False)
    y_d = nc.declare_dram_parameter("yT", [NVAR, L], f32, isOutput=True)

    with tile.TileContext(nc) as tc:
        with (
            tc.tile_pool(name="consts", bufs=1) as consts,
            tc.tile_pool(name="xgp", bufs=1) as xgp,
            tc.tile_pool(name="gpsum", bufs=2, space=bass.MemorySpace.PSUM) as gpsum,
            tc.tile_pool(name="dwp", bufs=3) as dwp,
            tc.tile_pool(name="prodp", bufs=3) as prodp,
            tc.tile_pool(name="opsum", bufs=2, space=bass.MemorySpace.PSUM) as opsum,
            tc.tile_pool(name="resp", bufs=2) as resp,
        ):
            NT = NCHUNK * ETILES  # 8 dw tiles
            HK = CHUNK * K // 2   # half-tile product columns (4096)
            HC = CHUNK // 2       # tau columns per half (256)

            # SP/HWDGE issue order = completion order (per-engine FIFO):
            # gather inputs first (small), then the dw stream owns the queue.
            # Tiles 0 and 7 are split into half-DMAs (16KB packets, slightly
            # slower) so the first multiply starts ~5us earlier and the tail
            # half overlaps its matmuls; middle tiles stay whole for peak
            # 32KB-packet bandwidth.
            xpad = consts.tile([NVAR, W_XPAD], f32r)
            nc.sync.dma_start(xpad[:], xpad_d[:])
            ssend = consts.tile([NVAR, E], f32r)
            nc.sync.dma_start(ssend[:], ssend_d[:])

            def dw_dma(dwt, ti, halves):
                h2, et = divmod(ti, ETILES)
                r0 = h2 * E + et * 128
                if halves:
                    for half in range(2):
                        nc.sync.dma_start(
                            dwt[:, half * HK:(half + 1) * HK],
                            dw_d[r0:r0 + 128, half * HK:(half + 1) * HK],
                        )
                else:
                    nc.sync.dma_start(dwt[:], dw_d[r0:r0 + 128, :])

            dwt_tiles = []
            for ti in range(NT):
                dwt = dwp.tile([128, CHUNK * K], f32, name="dwt", tag="dwt")
                dwt_tiles.append(dwt)
            dw_dma(dwt_tiles[0], 0, halves=True)
            # remaining small constants slot in behind the first dw tile
            wt = consts.tile([NVAR, K * NVAR], f32r)
            nc.sync.dma_start(wt[:], wt_d[:])
            recvT = consts.tile([128, ETILES * NVAR], bf16)
            nc.sync.dma_start(recvT[:], recv_d[:])
            bias_ones = consts.tile([1, NVAR + CHUNK], f32r)
            nc.sync.dma_start(bias_ones[:], bo_d[:])
            for ti in range(1, NT):
                dw_dma(dwt_tiles[ti], ti, halves=(ti == NT - 1))

            # Gather sender rows: xg[et][p, j] = xpad[send[et*128+p], j]
            xg = []
            for et in range(ETILES):
                xgt = xgp.tile([128, W_XPAD], f32, name=f"xg{et}", tag=f"xg{et}")
                for j0 in range(0, W_XPAD, CHUNK):
                    jw = min(CHUNK, W_XPAD - j0)
                    gps = gpsum.tile([128, CHUNK], f32, name="gps", tag="gps")
                    nc.tensor.matmul(
                        gps[:, :jw],
                        ssend[:, et * 128:(et + 1) * 128],
                        xpad[:, j0:j0 + jw],
                        start=True, stop=True,
                    )
                    nc.scalar.copy(xgt[:, j0:j0 + jw], gps[:, :jw])
                xg.append(xgt)

            ops_tiles = []
            for h2 in range(NCHUNK):
                o = opsum.tile([128, CHUNK], f32, name=f"ops{h2}", tag=f"ops{h2}")
                ops_tiles.append(o)

            def static_mm(h2, k, start=False):
                t0 = h2 * CHUNK
                nc.tensor.matmul(
                    ops_tiles[h2][:],
                    wt[:, k * NVAR:(k + 1) * NVAR],
                    xpad[:, t0 + k:t0 + k + CHUNK],
                    start=start, stop=False,
                )

            def bias_mm(h2):
                nc.tensor.matmul(
                    ops_tiles[h2][:],
                    bias_ones[:1, 0:NVAR],
                    bias_ones[:1, NVAR:NVAR + CHUNK],
                    start=False, stop=False,
                )

            # chunk-0 static conv + bias up front (PE warmup while dw streams)
            for k in range(K):
                static_mm(0, k, start=(k == 0))
            bias_mm(0)

            # chunk-1 static matmuls fill PE gaps across the first 7 groups
            fill = [("s", k) for k in range(K)] + [("b", None)]
            fills_per_group = [3, 3, 3, 2, 2, 2, 2, 0]

            KH = K // 2
            for ti in range(NT):
                h2, et = divmod(ti, ETILES)
                t0 = h2 * CHUNK
                dwt = dwt_tiles[ti]
                pt = prodp.tile([128, CHUNK * K], bf16, name="pt", tag="pt")
                drow = dwt.tensor.shape[-1]
                prow = pt.tensor.shape[-1]
                xrow = xg[et].tensor.shape[-1]
                # dw arrives k-major: dwt[e, k*CHUNK + tau]. Products keep that
                # layout, so every AP below is stride-1 in its innermost dim
                # (strided PE moving operands cost ~3-6 cycles/column, and
                # strided bf16 DVE writes hit sub-word read-modify-write).
                # Each tile is processed as two k-halves: the 8 matmuls of
                # half a run while DVE multiplies half b.
                for half in range(2):
                    k0 = half * KH
                    in0 = bass.AP(dwt.tensor, k0 * CHUNK,
                                  [[drow, 128], [CHUNK, KH], [1, CHUNK]])
                    # sliding window: in1[p, k, tau] = xg[p, t0 + tau + k]
                    in1 = bass.AP(xg[et].tensor, t0 + k0,
                                  [[xrow, 128], [1, KH], [1, CHUNK]])
                    out3 = bass.AP(pt.tensor, k0 * CHUNK,
                                   [[prow, 128], [CHUNK, KH], [1, CHUNK]])
                    nc.vector.tensor_mul(out3, in0, in1)
                    # k-reduction + recv scatter on PE (bf16, contiguous rhs):
                    # psum[v, tau] += sum_e recvT[e, v] * P[e, k*CHUNK + tau]
                    for k in range(k0, k0 + KH):
                        rhs = bass.AP(pt.tensor, k * CHUNK,
                                      [[prow, 128], [1, CHUNK]])
                        nc.tensor.matmul(
                            ops_tiles[h2][:],
                            recvT[:, et * NVAR:(et + 1) * NVAR],
                            rhs,
                            start=False,
                            stop=(et == ETILES - 1 and k == K - 1),
                        )
                for _ in range(fills_per_group[ti]):
                    kind, k = fill.pop(0)
                    if kind == "s":
                        static_mm(1, k, start=(k == 0))
                    else:
                        bias_mm(1)
                if et == ETILES - 1:
                    res = resp.tile([128, CHUNK], f32, name="res", tag="res")
                    nc.scalar.copy(res[:], ops_tiles[h2][:])
                    nc.gpsimd.dma_start(y_d[:, t0:t0 + CHUNK], res[:])

    nc.compile()
    return nc


def _get_program():
    global _PROGRAM
    if _PROGRAM is None:
        _PROGRAM = _build_program()
    return _PROGRAM


def _host_prep(spikes, conv_weight, conv_bias, dyn_weights, edge_send, edge_recv):
    spikes = np.asarray(spikes, dtype=np.float32)
    conv_weight = np.asarray(conv_weight, dtype=np.float32)
    conv_bias = np.asarray(conv_bias, dtype=np.float32)
    dyn_weights = np.asarray(dyn_weights, dtype=np.float32)
    edge_send = np.asarray(edge_send, dtype=np.int64)
    edge_recv = np.asarray(edge_recv, dtype=np.int64)

    x = np.ascontiguousarray(spikes[..., 0].transpose(0, 2, 1))  # [B, NVAR, T]

    ssend = np.zeros((NVAR, E), np.float32)
    ssend[edge_send, np.arange(E)] = 1.0

    import ml_dtypes
    recvT = np.zeros((128, ETILES * NVAR), ml_dtypes.bfloat16)
    for et in range(ETILES):
        rr = edge_recv[et * 128:(et + 1) * 128]
        recvT[np.arange(128), et * NVAR + rr] = 1.0

    w = conv_weight.copy()
    w[np.arange(NVAR), np.arange(NVAR), K - 1] = 0.0
    wt = np.ascontiguousarray(w.transpose(1, 2, 0)).reshape(NVAR, K * NVAR)

    bias_ones = np.concatenate(
        [conv_bias, np.ones(CHUNK, np.float32)]
    ).reshape(1, NVAR + CHUNK).astype(np.float32)

    in_maps = []
    for core in range(NC_COUNT):
        b, h = divmod(core, 2)
        tau0 = 0 if h == 0 else TAU - L  # 0 or 1023
        xpad = np.zeros((NVAR, W_XPAD), np.float32)
        lo = tau0 - (K - 2)  # first x column needed
        src_lo = max(lo, 0)
        xpad[:, src_lo - lo:W_XPAD - 1] = x[b, :, src_lo:tau0 + L + 1]
        a = dyn_weights[:, b, tau0:tau0 + L, :]          # [E, L, K]
        a = a.reshape(E, NCHUNK, CHUNK, K)               # [E, h2, tau, k]
        a = a.transpose(1, 0, 3, 2)                      # [h2, E, k, tau]
        dw = np.ascontiguousarray(a).reshape(NCHUNK * E, CHUNK * K)
        in_maps.append({
            "xpad": xpad,
            "dw": dw,
            "ssend": ssend,
            "wt": wt,
            "recvT": recvT,
            "bias_ones": bias_ones,
        })
    return in_maps


def _assemble(results):
    out = np.empty((B, TAU, NVAR, 1), np.float32)
    for core in range(NC_COUNT):
        b, h = divmod(core, 2)
        yT = results[core]["yT"]  # [NVAR, L]
        if h == 0:
            out[b, 0:L, :, 0] = yT.T
        else:
            out[b, L:TAU, :, 0] = yT[:, 1:L].T
    return out


def run_on_hw(in_maps, trace=False, **kwargs):
    from concourse.bass_utils import run_bass_kernel_spmd

    nc = _get_program()
    return run_bass_kernel_spmd(
        nc, in_maps, core_ids=list(range(NC_COUNT)), trace=trace, **kwargs
    )


def kernel(spikes, conv_weight, conv_bias, dyn_weights, edge_send, edge_recv):
    in_maps = _host_prep(
        spikes, conv_weight, conv_bias, dyn_weights, edge_send, edge_recv
    )
    res = run_on_hw(in_maps)
    return _assemble(res.results)



# revision 2
# speedup vs baseline: 1.6319x; 1.6319x over previous
"""Trainium2 Bass kernel for the spike-decoder GNN message-passing module.

Math (per batch b, output time tau in [0, T-2], variable v):
  out[b,tau,v] = bias[v]
               + sum_{i,k} w[v,i,k] * x[b,i,tau+k-(K-2)]          (static conv)
               + sum_{e: recv[e]=v} sum_k dw[e,b,tau,k] * x[b,send[e],tau+k-(K-2)]
with w = conv_weight masked at w[i,i,K-1] = 0, x = spikes[...,0] transposed to
[b, nvar, t], and out-of-range x treated as zero.

Sharding: 8 cores = (b in 0..3) x (time half h in 0..1). Each core computes a
1024-wide tau window ([0,1024) or [1023,2047) — one overlapping column keeps
shapes uniform for SPMD). dyn_weights is the memory-bound stream; it is cast
to bf16 on the host (tolerance 2e-2 dwarfs bf16's ~0.4% relative error),
halving the dominant HBM traffic to ~16.8 MB/core.

On-core algorithm:
  - xg[e,:] = x[send[e],:] is gathered on the HOST (pure indexing, like the
    one-hot recv matrix) and uploaded as bf16; a one-element-shifted copy
    xg_odd is made on ScalarE so every DVE sliding-window read starts 4B-
    aligned (bf16 2x perf mode needs aligned stride-{1,2} APs).
  - products P[e, k*CHUNK+tau] = dw * window(xg) on DVE, two tensor_muls per
    half-tile unit (even ks from xg, odd ks from xg_odd; both stride-2 in k).
  - k-reduction + recv-scatter + transpose folded into PE: per k, a matmul
    with stationary one-hot recvT and moving operand P[:, k*CHUNK+tau],
    accumulating into PSUM[v, tau].
  - static conv: 16 matmuls per tau-chunk with stationary wT_k (exact f32r)
    and shifted xpad slices, interleaved into PE gaps at unit boundaries.
  - bias: added by ScalarE during the PSUM -> SBUF copy (activation bias AP).
Output is [v, tau] per core; host transposes while assembling the result.
"""

import numpy as np

B, T, NVAR, K, E = 4, 2048, 128, 16, 512
TAU = T - 1            # 2047
L = 1024               # per-core tau window
NC_COUNT = 8
W_XPAD = L + K         # 1040
ETILES = E // 128      # 4
CHUNK = 512            # tau chunk per PSUM bank
NCHUNK = L // CHUNK    # 2
KH = K // 2            # 8 ks per half-tile unit
HK = KH * CHUNK        # 4096 product columns per unit
NT = NCHUNK * ETILES   # 8 dw tiles (et within h2)
NU = NT * 2            # 16 half-tile pipeline units

_PROGRAM = None


def _build_program():
    import concourse.bass as bass
    import concourse.bacc as bacc
    import concourse.mybir as mybir
    import concourse.tile as tile

    f32 = mybir.dt.float32
    f32r = mybir.dt.float32r
    bf16 = mybir.dt.bfloat16
    nc = bacc.Bacc()

    xpad_d = nc.declare_dram_parameter("xpad", [NVAR, W_XPAD], f32r, isOutput=False)
    xg_d = nc.declare_dram_parameter("xg", [E, W_XPAD], bf16, isOutput=False)
    dw_d = nc.declare_dram_parameter("dw", [NCHUNK * E, CHUNK * K], bf16, isOutput=False)
    wt_d = nc.declare_dram_parameter("wt", [NVAR, K * NVAR], f32r, isOutput=False)
    recv_d = nc.declare_dram_parameter("recvT", [128, ETILES * NVAR], bf16, isOutput=False)
    bias_d = nc.declare_dram_parameter("bias", [NVAR, 1], f32, isOutput=False)
    y_d = nc.declare_dram_parameter("yT", [NVAR, L], f32, isOutput=True)

    with tile.TileContext(nc) as tc:
        with (
            tc.tile_pool(name="consts", bufs=1) as consts,
            tc.tile_pool(name="dwp", bufs=5) as dwp,
            tc.tile_pool(name="prodp", bufs=3) as prodp,
            tc.tile_pool(name="opsum", bufs=2, space=bass.MemorySpace.PSUM) as opsum,
            tc.tile_pool(name="resp", bufs=2) as resp,
        ):
            # Persistent SBUF tensors. One HWDGE ring (sync) carries the whole
            # input stream in priority order; 349 GB/s sustained was measured
            # on this ring, so a second ring would only steal early bandwidth
            # from the critical first units.
            xg_e = []
            xg_o = []
            for et in range(ETILES):
                xg_e.append(consts.tile([128, W_XPAD], bf16, name=f"xge{et}"))
                xg_o.append(consts.tile([128, W_XPAD], bf16, name=f"xgo{et}"))
            xpad = consts.tile([NVAR, W_XPAD], f32r)
            wt = consts.tile([NVAR, K * NVAR], f32r)
            recvT = consts.tile([128, ETILES * NVAR], bf16)
            biasT = consts.tile([NVAR, 1], f32)

            dwt_tiles = []
            for u in range(NU):
                dwt_tiles.append(dwp.tile([128, HK], bf16, name="dwt", tag="dwt"))

            def dw_dma(u):
                ti, half = divmod(u, 2)
                h2, et = divmod(ti, ETILES)
                r0 = h2 * E + et * 128
                c0 = half * HK
                nc.sync.dma_start(dwt_tiles[u][:], dw_d[r0:r0 + 128, c0:c0 + HK])

            # DMA order: first xg0 (unit-0 products), then units 0/1 race with
            # the remaining constants, then the long dw stream.
            nc.sync.dma_start(xg_e[0][:], xg_d[0:128, :])
            dw_dma(0)
            nc.sync.dma_start(xpad[:], xpad_d[:])
            nc.sync.dma_start(xg_e[1][:], xg_d[128:256, :])
            dw_dma(1)
            nc.sync.dma_start(wt[:], wt_d[:])
            nc.sync.dma_start(xg_e[2][:], xg_d[256:384, :])
            nc.sync.dma_start(xg_e[3][:], xg_d[384:512, :])
            nc.sync.dma_start(recvT[:], recv_d[:])
            nc.sync.dma_start(biasT[:], bias_d[:])
            for u in range(2, NU):
                dw_dma(u)

            # Shifted copies for odd-k windows: xg_o[et][p, j] = xg_e[et][p, j+1]
            for et in range(ETILES):
                nc.scalar.copy(xg_o[et][:, 0:W_XPAD - 1], xg_e[et][:, 1:W_XPAD])

            ops_tiles = [
                opsum.tile([128, CHUNK], f32, name=f"ops{h2}", tag=f"ops{h2}")
                for h2 in range(NCHUNK)
            ]
            started = [False, False]

            def acc_mm(h2, lhsT, rhs, stop=False):
                st = not started[h2]
                started[h2] = True
                nc.tensor.matmul(ops_tiles[h2][:], lhsT, rhs, start=st, stop=stop)

            def static_mm(h2, k):
                t0 = h2 * CHUNK
                acc_mm(h2, wt[:, k * NVAR:(k + 1) * NVAR],
                       xpad[:, t0 + k:t0 + k + CHUNK])

            # Static-conv schedule: spread into PE gaps at unit boundaries,
            # starting once wt has arrived (queued behind unit 1).
            static_after = {u: [] for u in range(NU)}
            h0 = [(0, k) for k in range(K)]
            h1 = [(1, k) for k in range(K)]
            for i, s in enumerate(h0):
                static_after[3 + i % 4].append(s)
            for i, s in enumerate(h1):
                static_after[9 + i % 4].append(s)

            for u in range(NU):
                ti, half = divmod(u, 2)
                h2, et = divmod(ti, ETILES)
                t0 = h2 * CHUNK
                k0 = half * KH
                dwt = dwt_tiles[u]
                drow = dwt.tensor.shape[-1]
                pt = prodp.tile([128, HK], bf16, name="pt", tag="pt")
                prow = pt.tensor.shape[-1]
                xrow = xg_e[et].tensor.shape[-1]
                # Products for ks {k0..k0+7}, split by parity so every AP is
                # 4B-aligned with even strides (DVE 2x_1P bf16 mode).
                for par, xsrc in ((0, xg_e[et]), (1, xg_o[et])):
                    in0 = bass.AP(dwt.tensor, par * CHUNK,
                                  [[drow, 128], [2 * CHUNK, KH // 2], [1, CHUNK]])
                    # window: xsrc[p, t0 + k0 + 2m (+1 if odd, via xg_o) + tau]
                    in1 = bass.AP(xsrc.tensor, t0 + k0,
                                  [[xrow, 128], [2, KH // 2], [1, CHUNK]])
                    out3 = bass.AP(pt.tensor, par * CHUNK,
                                   [[prow, 128], [2 * CHUNK, KH // 2], [1, CHUNK]])
                    nc.vector.tensor_mul(out3, in0, in1)
                # k-reduction + recv scatter on PE:
                # psum[v, tau] += sum_e recvT[e, v] * P[e, kl*CHUNK + tau]
                last_unit_of_h2 = (u == NT - 1 or u == NU - 1)
                for kl in range(KH):
                    rhs = bass.AP(pt.tensor, kl * CHUNK, [[prow, 128], [1, CHUNK]])
                    acc_mm(h2, recvT[:, et * NVAR:(et + 1) * NVAR], rhs,
                           stop=(last_unit_of_h2 and kl == KH - 1
                                 and not static_after[u]))
                for h2s, k in static_after[u]:
                    static_mm(h2s, k)
                if last_unit_of_h2:
                    res = resp.tile([128, CHUNK], f32, name="res", tag="res")
                    # copy-out with bias folded in: res = ops + bias[v]
                    nc.scalar.add(res[:], ops_tiles[h2][:], biasT[:, 0:1])
                    nc.gpsimd.dma_start(y_d[:, t0:t0 + CHUNK], res[:])

    nc.compile()
    return nc


def _get_program():
    global _PROGRAM
    if _PROGRAM is None:
        _PROGRAM = _build_program()
    return _PROGRAM


def _host_prep(spikes, conv_weight, conv_bias, dyn_weights, edge_send, edge_recv):
    import ml_dtypes

    spikes = np.asarray(spikes, dtype=np.float32)
    conv_weight = np.asarray(conv_weight, dtype=np.float32)
    conv_bias = np.asarray(conv_bias, dtype=np.float32)
    dyn_weights = np.asarray(dyn_weights, dtype=np.float32)
    edge_send = np.asarray(edge_send, dtype=np.int64)
    edge_recv = np.asarray(edge_recv, dtype=np.int64)

    x = np.ascontiguousarray(spikes[..., 0].transpose(0, 2, 1))  # [B, NVAR, T]

    recvT = np.zeros((128, ETILES * NVAR), ml_dtypes.bfloat16)
    for et in range(ETILES):
        rr = edge_recv[et * 128:(et + 1) * 128]
        recvT[np.arange(128), et * NVAR + rr] = 1.0

    w = conv_weight.copy()
    w[np.arange(NVAR), np.arange(NVAR), K - 1] = 0.0
    wt = np.ascontiguousarray(w.transpose(1, 2, 0)).reshape(NVAR, K * NVAR)

    bias_col = np.ascontiguousarray(conv_bias.reshape(NVAR, 1))

    in_maps = []
    for core in range(NC_COUNT):
        b, h = divmod(core, 2)
        tau0 = 0 if h == 0 else TAU - L  # 0 or 1023
        xpad = np.zeros((NVAR, W_XPAD), np.float32)
        lo = tau0 - (K - 2)  # first x column needed
        src_lo = max(lo, 0)
        xpad[:, src_lo - lo:W_XPAD - 1] = x[b, :, src_lo:tau0 + L + 1]
        xg = np.ascontiguousarray(xpad[edge_send, :]).astype(ml_dtypes.bfloat16)
        a = dyn_weights[:, b, tau0:tau0 + L, :]          # [E, L, K]
        a = a.reshape(E, NCHUNK, CHUNK, K)               # [E, h2, tau, k]
        a = a.transpose(1, 0, 3, 2)                      # [h2, E, k, tau]
        dw = np.ascontiguousarray(a).reshape(NCHUNK * E, CHUNK * K)
        dw = dw.astype(ml_dtypes.bfloat16)
        in_maps.append({
            "xpad": xpad,
            "xg": xg,
            "dw": dw,
            "wt": wt,
            "recvT": recvT,
            "bias": bias_col,
        })
    return in_maps


def _assemble(results):
    out = np.empty((B, TAU, NVAR, 1), np.float32)
    for core in range(NC_COUNT):
        b, h = divmod(core, 2)
        yT = results[core]["yT"]  # [NVAR, L]
        if h == 0:
            out[b, 0:L, :, 0] = yT.T
        else:
            out[b, L:TAU, :, 0] = yT[:, 1:L].T
    return out


def run_on_hw(in_maps, trace=False, **kwargs):
    from concourse.bass_utils import run_bass_kernel_spmd

    nc = _get_program()
    return run_bass_kernel_spmd(
        nc, in_maps, core_ids=list(range(NC_COUNT)), trace=trace, **kwargs
    )


def kernel(spikes, conv_weight, conv_bias, dyn_weights, edge_send, edge_recv):
    in_maps = _host_prep(
        spikes, conv_weight, conv_bias, dyn_weights, edge_send, edge_recv
    )
    res = run_on_hw(in_maps)
    return _assemble(res.results)


# revision 4
# speedup vs baseline: 1.6564x; 1.0150x over previous
"""Trainium2 Bass kernel for the spike-decoder GNN message-passing module.

Math (per batch b, output time tau in [0, T-2], variable v):
  out[b,tau,v] = bias[v]
               + sum_{i,k} w[v,i,k] * x[b,i,tau+k-(K-2)]          (static conv)
               + sum_{e: recv[e]=v} sum_k dw[e,b,tau,k] * x[b,send[e],tau+k-(K-2)]
with w = conv_weight masked at w[i,i,K-1] = 0, x = spikes[...,0] transposed to
[b, nvar, t], and out-of-range x treated as zero.

Sharding: 8 cores = (b in 0..3) x (time half h in 0..1). Each core computes a
1024-wide tau window ([0,1024) or [1023,2047) — one overlapping column keeps
shapes uniform for SPMD). dyn_weights is the memory-bound stream; it is cast
to bf16 on the host (tolerance 2e-2 dwarfs bf16's ~0.4% relative error),
halving the dominant HBM traffic to ~16.8 MB/core.

On-core algorithm:
  - xg[e,:] = x[send[e],:] is gathered on the HOST (pure indexing, like the
    one-hot recv matrix) and uploaded as bf16; a one-element-shifted copy
    xg_odd is made on ScalarE so every DVE sliding-window read starts 4B-
    aligned (bf16 2x perf mode needs aligned stride-{1,2} APs).
  - products P[e, k*CHUNK+tau] = dw * window(xg) on DVE, two tensor_muls per
    half-tile unit (even ks from xg, odd ks from xg_odd; both stride-2 in k).
  - k-reduction + recv-scatter + transpose folded into PE: per k, a matmul
    with stationary one-hot recvT and moving operand P[:, k*CHUNK+tau],
    accumulating into PSUM[v, tau].
  - static conv: 16 matmuls per tau-chunk with stationary wT_k (exact f32r)
    and shifted xpad slices, interleaved into PE gaps at unit boundaries.
  - bias: added by ScalarE during the PSUM -> SBUF copy (activation bias AP).
Output is [v, tau] per core; host transposes while assembling the result.
"""

import numpy as np

B, T, NVAR, K, E = 4, 2048, 128, 16, 512
TAU = T - 1            # 2047
L = 1024               # per-core tau window
NC_COUNT = 8
W_XPAD = L + K         # 1040
ETILES = E // 128      # 4
CHUNK = 512            # tau chunk per PSUM bank
NCHUNK = L // CHUNK    # 2
KH = K // 2            # 8 ks per half-tile unit
HK = KH * CHUNK        # 4096 product columns per unit
NT = NCHUNK * ETILES   # 8 dw tiles (et within h2)
NU = NT * 2            # 16 half-tile pipeline units

_PROGRAM = None


def _build_program():
    import concourse.bass as bass
    import concourse.bacc as bacc
    import concourse.mybir as mybir
    import concourse.tile as tile

    f32 = mybir.dt.float32
    f32r = mybir.dt.float32r
    bf16 = mybir.dt.bfloat16
    nc = bacc.Bacc()

    xpad_d = nc.declare_dram_parameter("xpad", [NVAR, W_XPAD], f32r, isOutput=False)
    xg_d = nc.declare_dram_parameter("xg", [E, W_XPAD], bf16, isOutput=False)
    dw_d = nc.declare_dram_parameter("dw", [NCHUNK * E, CHUNK * K], bf16, isOutput=False)
    wt_d = nc.declare_dram_parameter("wt", [NVAR, K * NVAR], f32r, isOutput=False)
    recv_d = nc.declare_dram_parameter("recvT", [128, ETILES * NVAR], bf16, isOutput=False)
    bias_d = nc.declare_dram_parameter("bias", [NVAR, 1], f32, isOutput=False)
    y_d = nc.declare_dram_parameter("yT", [NVAR, L], f32, isOutput=True)

    with tile.TileContext(nc) as tc:
        with (
            tc.tile_pool(name="consts", bufs=1) as consts,
            tc.tile_pool(name="dwp", bufs=6) as dwp,
            tc.tile_pool(name="prodp", bufs=4) as prodp,
            tc.tile_pool(name="opsum", bufs=2, space=bass.MemorySpace.PSUM) as opsum,
            tc.tile_pool(name="resp", bufs=2) as resp,
        ):
            # Persistent SBUF tensors. One HWDGE ring (sync) carries the whole
            # input stream in priority order; 349 GB/s sustained was measured
            # on this ring, so a second ring would only steal early bandwidth
            # from the critical first units.
            xg_e = []
            xg_o = []
            for et in range(ETILES):
                xg_e.append(consts.tile([128, W_XPAD], bf16, name=f"xge{et}"))
                xg_o.append(consts.tile([128, W_XPAD], bf16, name=f"xgo{et}"))
            xpad = consts.tile([NVAR, W_XPAD], f32r)
            wt = consts.tile([NVAR, K * NVAR], f32r)
            recvT = consts.tile([128, ETILES * NVAR], bf16)
            biasT = consts.tile([NVAR, 1], f32)

            dwt_tiles = []
            for u in range(NU):
                dwt_tiles.append(dwp.tile([128, HK], bf16, name="dwt", tag="dwt"))

            def dw_dma(u):
                ti, half = divmod(u, 2)
                h2, et = divmod(ti, ETILES)
                r0 = h2 * E + et * 128
                c0 = half * HK
                nc.sync.dma_start(dwt_tiles[u][:], dw_d[r0:r0 + 128, c0:c0 + HK])

            # DMA order: every operand of unit-0's first matmul chain (recvT,
            # xg0, dw0) leads the queue — the first dyn MM fires as soon as
            # the first products exist. Static-conv operands (xpad, wt) ride
            # behind the first two dw units; their matmuls only start at unit
            # 3 boundaries.
            nc.sync.dma_start(recvT[:], recv_d[:])
            nc.sync.dma_start(biasT[:], bias_d[:])
            nc.sync.dma_start(xg_e[0][:], xg_d[0:128, :])
            dw_dma(0)
            nc.sync.dma_start(xg_e[1][:], xg_d[128:256, :])
            dw_dma(1)
            nc.sync.dma_start(xpad[:], xpad_d[:])
            nc.sync.dma_start(wt[:], wt_d[:])
            nc.sync.dma_start(xg_e[2][:], xg_d[256:384, :])
            nc.sync.dma_start(xg_e[3][:], xg_d[384:512, :])
            for u in range(2, NU):
                dw_dma(u)

            # Shifted copies for odd-k windows: xg_o[et][p, j] = xg_e[et][p, j+1]
            for et in range(ETILES):
                nc.scalar.copy(xg_o[et][:, 0:W_XPAD - 1], xg_e[et][:, 1:W_XPAD])

            ops_tiles = [
                opsum.tile([128, CHUNK], f32, name=f"ops{h2}", tag=f"ops{h2}")
                for h2 in range(NCHUNK)
            ]
            started = [False, False]

            def acc_mm(h2, lhsT, rhs, stop=False):
                st = not started[h2]
                started[h2] = True
                nc.tensor.matmul(ops_tiles[h2][:], lhsT, rhs, start=st, stop=stop)

            def static_mm(h2, k):
                t0 = h2 * CHUNK
                acc_mm(h2, wt[:, k * NVAR:(k + 1) * NVAR],
                       xpad[:, t0 + k:t0 + k + CHUNK])

            # Static-conv schedule: spread into PE gaps at unit boundaries,
            # starting once wt has arrived (queued behind unit 1).
            static_after = {u: [] for u in range(NU)}
            h0 = [(0, k) for k in range(K)]
            h1 = [(1, k) for k in range(K)]
            for i, s in enumerate(h0):
                static_after[3 + i % 4].append(s)
            for i, s in enumerate(h1):
                static_after[9 + i % 4].append(s)

            for u in range(NU):
                ti, half = divmod(u, 2)
                h2, et = divmod(ti, ETILES)
                t0 = h2 * CHUNK
                k0 = half * KH
                dwt = dwt_tiles[u]
                drow = dwt.tensor.shape[-1]
                pt = prodp.tile([128, HK], bf16, name="pt", tag="pt")
                prow = pt.tensor.shape[-1]
                xrow = xg_e[et].tensor.shape[-1]
                # Products for ks {k0..k0+7}, split by parity so every AP is
                # 4B-aligned with even strides (DVE 2x_1P bf16 mode).
                for par, xsrc in ((0, xg_e[et]), (1, xg_o[et])):
                    in0 = bass.AP(dwt.tensor, par * CHUNK,
                                  [[drow, 128], [2 * CHUNK, KH // 2], [1, CHUNK]])
                    # window: xsrc[p, t0 + k0 + 2m (+1 if odd, via xg_o) + tau]
                    in1 = bass.AP(xsrc.tensor, t0 + k0,
                                  [[xrow, 128], [2, KH // 2], [1, CHUNK]])
                    out3 = bass.AP(pt.tensor, par * CHUNK,
                                   [[prow, 128], [2 * CHUNK, KH // 2], [1, CHUNK]])
                    nc.vector.tensor_mul(out3, in0, in1)
                # k-reduction + recv scatter on PE:
                # psum[v, tau] += sum_e recvT[e, v] * P[e, kl*CHUNK + tau]
                last_unit_of_h2 = (u == NT - 1 or u == NU - 1)
                for kl in range(KH):
                    rhs = bass.AP(pt.tensor, kl * CHUNK, [[prow, 128], [1, CHUNK]])
                    acc_mm(h2, recvT[:, et * NVAR:(et + 1) * NVAR], rhs,
                           stop=(last_unit_of_h2 and kl == KH - 1
                                 and not static_after[u]))
                for h2s, k in static_after[u]:
                    static_mm(h2s, k)
                if last_unit_of_h2:
                    res = resp.tile([128, CHUNK], f32, name="res", tag="res")
                    # copy-out with bias folded in: res = ops + bias[v]
                    nc.scalar.add(res[:], ops_tiles[h2][:], biasT[:, 0:1])
                    nc.gpsimd.dma_start(y_d[:, t0:t0 + CHUNK], res[:])

    nc.compile()
    return nc


def _get_program():
    global _PROGRAM
    if _PROGRAM is None:
        _PROGRAM = _build_program()
    return _PROGRAM


def _host_prep(spikes, conv_weight, conv_bias, dyn_weights, edge_send, edge_recv):
    import ml_dtypes

    spikes = np.asarray(spikes, dtype=np.float32)
    conv_weight = np.asarray(conv_weight, dtype=np.float32)
    conv_bias = np.asarray(conv_bias, dtype=np.float32)
    dyn_weights = np.asarray(dyn_weights, dtype=np.float32)
    edge_send = np.asarray(edge_send, dtype=np.int64)
    edge_recv = np.asarray(edge_recv, dtype=np.int64)

    x = np.ascontiguousarray(spikes[..., 0].transpose(0, 2, 1))  # [B, NVAR, T]

    recvT = np.zeros((128, ETILES * NVAR), ml_dtypes.bfloat16)
    for et in range(ETILES):
        rr = edge_recv[et * 128:(et + 1) * 128]
        recvT[np.arange(128), et * NVAR + rr] = 1.0

    w = conv_weight.copy()
    w[np.arange(NVAR), np.arange(NVAR), K - 1] = 0.0
    wt = np.ascontiguousarray(w.transpose(1, 2, 0)).reshape(NVAR, K * NVAR)

    bias_col = np.ascontiguousarray(conv_bias.reshape(NVAR, 1))

    in_maps = []
    for core in range(NC_COUNT):
        b, h = divmod(core, 2)
        tau0 = 0 if h == 0 else TAU - L  # 0 or 1023
        xpad = np.zeros((NVAR, W_XPAD), np.float32)
        lo = tau0 - (K - 2)  # first x column needed
        src_lo = max(lo, 0)
        xpad[:, src_lo - lo:W_XPAD - 1] = x[b, :, src_lo:tau0 + L + 1]
        xg = np.ascontiguousarray(xpad[edge_send, :]).astype(ml_dtypes.bfloat16)
        a = dyn_weights[:, b, tau0:tau0 + L, :]          # [E, L, K]
        a = a.reshape(E, NCHUNK, CHUNK, K)               # [E, h2, tau, k]
        a = a.transpose(1, 0, 3, 2)                      # [h2, E, k, tau]
        dw = np.ascontiguousarray(a).reshape(NCHUNK * E, CHUNK * K)
        dw = dw.astype(ml_dtypes.bfloat16)
        in_maps.append({
            "xpad": xpad,
            "xg": xg,
            "dw": dw,
            "wt": wt,
            "recvT": recvT,
            "bias": bias_col,
        })
    return in_maps


def _assemble(results):
    out = np.empty((B, TAU, NVAR, 1), np.float32)
    for core in range(NC_COUNT):
        b, h = divmod(core, 2)
        yT = results[core]["yT"]  # [NVAR, L]
        if h == 0:
            out[b, 0:L, :, 0] = yT.T
        else:
            out[b, L:TAU, :, 0] = yT[:, 1:L].T
    return out


def run_on_hw(in_maps, trace=False, **kwargs):
    from concourse.bass_utils import run_bass_kernel_spmd

    nc = _get_program()
    return run_bass_kernel_spmd(
        nc, in_maps, core_ids=list(range(NC_COUNT)), trace=trace, **kwargs
    )


def kernel(spikes, conv_weight, conv_bias, dyn_weights, edge_send, edge_recv):
    in_maps = _host_prep(
        spikes, conv_weight, conv_bias, dyn_weights, edge_send, edge_recv
    )
    res = run_on_hw(in_maps)
    return _assemble(res.results)


# revision 5
# speedup vs baseline: 1.7480x; 1.0553x over previous
"""Trainium2 Bass kernel for the spike-decoder GNN message-passing module.

Math (per batch b, output time tau in [0, T-2], variable v):
  out[b,tau,v] = bias[v]
               + sum_{i,k} w[v,i,k] * x[b,i,tau+k-(K-2)]          (static conv)
               + sum_{e: recv[e]=v} sum_k dw[e,b,tau,k] * x[b,send[e],tau+k-(K-2)]
with w = conv_weight masked at w[i,i,K-1] = 0, x = spikes[...,0] transposed to
[b, nvar, t], and out-of-range x treated as zero.

Sharding: 8 cores = (b in 0..3) x (time half h in 0..1). Each core computes a
1024-wide tau window ([0,1024) or [1023,2047) — one overlapping column keeps
shapes uniform for SPMD). dyn_weights is the memory-bound stream; it is cast
to bf16 on the host (tolerance 2e-2 dwarfs bf16's ~0.4% relative error),
halving the dominant HBM traffic to ~16.8 MB/core.

On-core algorithm:
  - xg[e,:] = x[send[e],:] is gathered on the HOST (pure indexing, like the
    one-hot recv matrix) and uploaded as bf16; a one-element-shifted copy
    xg_odd is made on ScalarE so every DVE sliding-window read starts 4B-
    aligned (bf16 2x perf mode needs aligned stride-{1,2} APs).
  - the dw stream is laid out by the host as 32 parity blocks per core
    (unit u = 8 consecutive ks of one (h2, et) tile; block A = even ks,
    block B = odd ks, each [128, 4*512] bf16 = 512 KB). Each block is one
    DMA and gates exactly one DVE tensor_mul — fine-grained DMA->DVE->PE
    pipelining with ~1.3 us per stage.
  - products P[e, m*CHUNK+tau] = dw_block * window(xg) on DVE (2x bf16).
  - k-reduction + recv-scatter + transpose folded into PE: per product
    column block, a matmul with stationary one-hot recvT accumulating into
    PSUM[v, tau].
  - static conv: 16 matmuls per tau-chunk with stationary wT_k (bf16) and
    shifted xpad slices (parity copies for alignment), interleaved into PE
    gaps at unit boundaries mid-stream.
  - bias: added by ScalarE during the PSUM -> SBUF copy (activation bias AP).
Output is [v, tau] per core; host transposes while assembling the result.
"""

import numpy as np

B, T, NVAR, K, E = 4, 2048, 128, 16, 512
TAU = T - 1            # 2047
L = 1024               # per-core tau window
NC_COUNT = 8
W_XPAD = L + K         # 1040
ETILES = E // 128      # 4
CHUNK = 512            # tau chunk per PSUM bank
NCHUNK = L // CHUNK    # 2
KH = K // 2            # 8 ks per half-tile unit
KQ = KH // 2           # 4 ks per parity block
BLK = KQ * CHUNK       # 2048 product columns per parity block
NT = NCHUNK * ETILES   # 8 dw tiles (et within h2)
NU = NT * 2            # 16 half-tile pipeline units

# host-side k reordering within each 8-k half: evens then odds
K_ORDER = [0, 2, 4, 6, 1, 3, 5, 7, 8, 10, 12, 14, 9, 11, 13, 15]

_PROGRAM = None


def _build_program():
    import concourse.bass as bass
    import concourse.bacc as bacc
    import concourse.mybir as mybir
    import concourse.tile as tile

    f32 = mybir.dt.float32
    bf16 = mybir.dt.bfloat16
    nc = bacc.Bacc()

    xpad_d = nc.declare_dram_parameter("xpad", [NVAR, W_XPAD], bf16, isOutput=False)
    xg_d = nc.declare_dram_parameter("xg", [E, W_XPAD], bf16, isOutput=False)
    dw_d = nc.declare_dram_parameter("dw", [NCHUNK * E, CHUNK * K], bf16, isOutput=False)
    wt_d = nc.declare_dram_parameter("wt", [NVAR, K * NVAR], bf16, isOutput=False)
    recv_d = nc.declare_dram_parameter("recvT", [128, ETILES * NVAR], bf16, isOutput=False)
    bias_d = nc.declare_dram_parameter("bias", [NVAR, 1], f32, isOutput=False)
    y_d = nc.declare_dram_parameter("yT", [NVAR, L], f32, isOutput=True)

    with tile.TileContext(nc) as tc:
        with (
            tc.tile_pool(name="consts", bufs=1) as consts,
            tc.tile_pool(name="dwp", bufs=12) as dwp,
            tc.tile_pool(name="prodp", bufs=8) as prodp,
            tc.tile_pool(name="opsum", bufs=2, space=bass.MemorySpace.PSUM) as opsum,
            tc.tile_pool(name="resp", bufs=2) as resp,
        ):
            xg_e = []
            xg_o = []
            for et in range(ETILES):
                xg_e.append(consts.tile([128, W_XPAD], bf16, name=f"xge{et}"))
                xg_o.append(consts.tile([128, W_XPAD], bf16, name=f"xgo{et}"))
            xpad_e = consts.tile([NVAR, W_XPAD], bf16)
            xpad_o = consts.tile([NVAR, W_XPAD], bf16)
            wt = consts.tile([NVAR, K * NVAR], bf16)
            recvT = consts.tile([128, ETILES * NVAR], bf16)
            biasT = consts.tile([NVAR, 1], f32)

            # 32 parity blocks; pool rotation (bufs=12) provides ~6 units of
            # DMA runway ahead of compute.
            blk_tiles = [dwp.tile([128, BLK], bf16, name="blk", tag="blk")
                         for _ in range(2 * NU)]

            def blk_dma(u, par):
                ti, half = divmod(u, 2)
                h2, et = divmod(ti, ETILES)
                r0 = h2 * E + et * 128
                c0 = half * 2 * BLK + par * BLK
                nc.sync.dma_start(blk_tiles[2 * u + par][:],
                                  dw_d[r0:r0 + 128, c0:c0 + BLK])

            # DMA order: unit-0's matmul operands first (recvT, xg0, blocks),
            # then dw stream with the remaining consts slotted in early enough
            # for their consumers but without starving the stream.
            nc.sync.dma_start(recvT[:], recv_d[:])
            nc.sync.dma_start(xg_e[0][:], xg_d[0:128, :])
            blk_dma(0, 0)
            blk_dma(0, 1)
            nc.sync.dma_start(xg_e[1][:], xg_d[128:256, :])
            blk_dma(1, 0)
            blk_dma(1, 1)
            blk_dma(2, 0)
            blk_dma(2, 1)
            nc.sync.dma_start(xg_e[2][:], xg_d[256:384, :])
            nc.sync.dma_start(xg_e[3][:], xg_d[384:512, :])
            blk_dma(3, 0)
            blk_dma(3, 1)
            nc.sync.dma_start(xpad_e[:], xpad_d[:])
            nc.sync.dma_start(wt[:], wt_d[:])
            nc.sync.dma_start(biasT[:], bias_d[:])
            for u in range(4, NU):
                blk_dma(u, 0)
                blk_dma(u, 1)

            # Shifted copies for odd-k windows (ScalarE, otherwise idle):
            # xg_o[et][p, j] = xg_e[et][p, j+1]; same for xpad.
            for et in range(ETILES):
                nc.scalar.copy(xg_o[et][:, 0:W_XPAD - 1], xg_e[et][:, 1:W_XPAD])
            nc.scalar.copy(xpad_o[:, 0:W_XPAD - 1], xpad_e[:, 1:W_XPAD])

            ops_tiles = [
                opsum.tile([128, CHUNK], f32, name=f"ops{h2}", tag=f"ops{h2}")
                for h2 in range(NCHUNK)
            ]
            started = [False, False]

            def acc_mm(h2, lhsT, rhs, stop=False):
                st = not started[h2]
                started[h2] = True
                nc.tensor.matmul(ops_tiles[h2][:], lhsT, rhs, start=st, stop=stop)

            def static_mm(h2, k):
                t0 = h2 * CHUNK
                p = k & 1
                src = xpad_o if p else xpad_e
                acc_mm(h2, wt[:, k * NVAR:(k + 1) * NVAR],
                       src[:, t0 + k - p:t0 + k - p + CHUNK])

            # Static-conv schedule: into PE gaps at unit boundaries once
            # wt/xpad have arrived (queued behind unit 3), finishing before
            # each half's copy-out.
            static_after = {u: [] for u in range(NU)}
            for i, k in enumerate(range(K)):
                static_after[5 + i % 2].append((0, k))
            for i, k in enumerate(range(K)):
                static_after[9 + i % 4].append((1, k))

            for u in range(NU):
                ti, half = divmod(u, 2)
                h2, et = divmod(ti, ETILES)
                t0 = h2 * CHUNK
                k0 = half * KH
                last_unit_of_h2 = (u == NT - 1 or u == NU - 1)
                for par, xsrc in ((0, xg_e[et]), (1, xg_o[et])):
                    blk = blk_tiles[2 * u + par]
                    brow = blk.tensor.shape[-1]
                    pt = prodp.tile([128, BLK], bf16, name="pt", tag="pt")
                    prow = pt.tensor.shape[-1]
                    xrow = xsrc.tensor.shape[-1]
                    in0 = bass.AP(blk.tensor, 0,
                                  [[brow, 128], [CHUNK, KQ], [1, CHUNK]])
                    # window: xsrc[p, t0 + k0 + 2m (+1 via xg_o) + tau]
                    in1 = bass.AP(xsrc.tensor, t0 + k0,
                                  [[xrow, 128], [2, KQ], [1, CHUNK]])
                    out3 = bass.AP(pt.tensor, 0,
                                   [[prow, 128], [CHUNK, KQ], [1, CHUNK]])
                    nc.vector.tensor_mul(out3, in0, in1)
                    # k-reduction + recv scatter on PE:
                    # psum[v, tau] += sum_e recvT[e, v] * P[e, m*CHUNK + tau]
                    for m in range(KQ):
                        rhs = bass.AP(pt.tensor, m * CHUNK,
                                      [[prow, 128], [1, CHUNK]])
                        acc_mm(h2, recvT[:, et * NVAR:(et + 1) * NVAR], rhs,
                               stop=(last_unit_of_h2 and par == 1
                                     and m == KQ - 1 and not static_after[u]))
                for h2s, k in static_after[u]:
                    static_mm(h2s, k)
                if last_unit_of_h2:
                    res = resp.tile([128, CHUNK], f32, name="res", tag="res")
                    # copy-out with bias folded in: res = ops + bias[v]
                    nc.scalar.add(res[:], ops_tiles[h2][:], biasT[:, 0:1])
                    nc.gpsimd.dma_start(y_d[:, t0:t0 + CHUNK], res[:])

    nc.compile()
    return nc


def _get_program():
    global _PROGRAM
    if _PROGRAM is None:
        _PROGRAM = _build_program()
    return _PROGRAM


def _host_prep(spikes, conv_weight, conv_bias, dyn_weights, edge_send, edge_recv):
    import ml_dtypes

    spikes = np.asarray(spikes, dtype=np.float32)
    conv_weight = np.asarray(conv_weight, dtype=np.float32)
    conv_bias = np.asarray(conv_bias, dtype=np.float32)
    dyn_weights = np.asarray(dyn_weights, dtype=np.float32)
    edge_send = np.asarray(edge_send, dtype=np.int64)
    edge_recv = np.asarray(edge_recv, dtype=np.int64)

    x = np.ascontiguousarray(spikes[..., 0].transpose(0, 2, 1))  # [B, NVAR, T]

    recvT = np.zeros((128, ETILES * NVAR), ml_dtypes.bfloat16)
    for et in range(ETILES):
        rr = edge_recv[et * 128:(et + 1) * 128]
        recvT[np.arange(128), et * NVAR + rr] = 1.0

    w = conv_weight.copy()
    w[np.arange(NVAR), np.arange(NVAR), K - 1] = 0.0
    wt = np.ascontiguousarray(w.transpose(1, 2, 0)).reshape(NVAR, K * NVAR)
    wt = wt.astype(ml_dtypes.bfloat16)

    bias_col = np.ascontiguousarray(conv_bias.reshape(NVAR, 1))

    in_maps = []
    for core in range(NC_COUNT):
        b, h = divmod(core, 2)
        tau0 = 0 if h == 0 else TAU - L  # 0 or 1023
        xpad = np.zeros((NVAR, W_XPAD), np.float32)
        lo = tau0 - (K - 2)  # first x column needed
        src_lo = max(lo, 0)
        xpad[:, src_lo - lo:W_XPAD - 1] = x[b, :, src_lo:tau0 + L + 1]
        xg = np.ascontiguousarray(xpad[edge_send, :]).astype(ml_dtypes.bfloat16)
        a = dyn_weights[:, b, tau0:tau0 + L, :]          # [E, L, K]
        a = a.reshape(E, NCHUNK, CHUNK, K)               # [E, h2, tau, k]
        a = a.transpose(1, 0, 3, 2)                      # [h2, E, k, tau]
        a = a[:, :, K_ORDER, :]                          # parity-block k order
        dw = np.ascontiguousarray(a).reshape(NCHUNK * E, CHUNK * K)
        dw = dw.astype(ml_dtypes.bfloat16)
        in_maps.append({
            "xpad": xpad.astype(ml_dtypes.bfloat16),
            "xg": xg,
            "dw": dw,
            "wt": wt,
            "recvT": recvT,
            "bias": bias_col,
        })
    return in_maps


def _assemble(results):
    out = np.empty((B, TAU, NVAR, 1), np.float32)
    for core in range(NC_COUNT):
        b, h = divmod(core, 2)
        yT = results[core]["yT"]  # [NVAR, L]
        if h == 0:
            out[b, 0:L, :, 0] = yT.T
        else:
            out[b, L:TAU, :, 0] = yT[:, 1:L].T
    return out


def run_on_hw(in_maps, trace=False, **kwargs):
    from concourse.bass_utils import run_bass_kernel_spmd

    nc = _get_program()
    return run_bass_kernel_spmd(
        nc, in_maps, core_ids=list(range(NC_COUNT)), trace=trace, **kwargs
    )


def kernel(spikes, conv_weight, conv_bias, dyn_weights, edge_send, edge_recv):
    in_maps = _host_prep(
        spikes, conv_weight, conv_bias, dyn_weights, edge_send, edge_recv
    )
    res = run_on_hw(in_maps)
    return _assemble(res.results)
